# revision 1
# baseline (speedup 1.0000x reference)
"""AXSConv2d (6-bit block fake-quant 3x3 conv, stride 1, pad 1) on 8 trn2
NeuronCores.

Sharding: data-parallel over batch (32 images -> 4 per core); dequantized
weight + bias replicated on every core.

Device algorithm (per core):
  - Input x is zero-padded on host to [4, 256, 58, 58] so that the conv
    becomes 9 constant-offset shifted matmuls over the flattened padded
    plane (3364 = 58*58 positions).
  - SBUF holds x as fp16 [ci(128) x (chunk, n, 3364)] (2 ci-chunks).
  - For each (n, co_tile of 128, spatial chunk of <=464 cols): accumulate
    18 matmuls (2 ci-chunks x 9 taps) into one PSUM bank, evict with a
    per-partition bias add (DVE) into an SBUF staging tile, then DMA the
    valid 56x56 interior back to HBM.
  - Compute dtype fp16 (PSUM accumulation fp32): ~3e-4 rel err vs fp32
    reference, full TensorE rate.

The weight fake-quant (axs6) is computed on host in fp32 — bit-identical
to the reference jax math — so device error is matmul rounding only.
"""
import sys

import numpy as np

for _p in ("/opt/trn_rl_repo", "/opt/pypackages"):
    if _p not in sys.path:
        sys.path.append(_p)

import concourse.bass as bass  # noqa: E402
import concourse.mybir as mybir  # noqa: E402
from concourse import bacc  # noqa: E402
from concourse.tile import TileContext  # noqa: E402
from concourse.bass_utils import run_bass_kernel_spmd  # noqa: E402

# Problem shape (hardcoded: the harness contract for this problem)
N_CORES = 8
B, CIN, COUT, H, W = 32, 256, 512, 56, 56
KH = KW = 3
BP = B // N_CORES          # batch per core
HP, WP = H + 2, W + 2      # padded plane
PLANE = HP * WP            # 3364
NCHUNK = CIN // 128        # ci chunks of 128
NQ = COUT // 128           # co tiles of 128
OUT_SPAN = H * WP          # 3248: flat positions [59, 59+3248) hold all outputs
CHUNK_W = 464              # matmul moving width (<= 512 fp32 psum bank)
N_K = 7                    # ceil(3246 / 464): 6 x 464 + 462
LAST_W = (OUT_SPAN - 2) - (N_K - 1) * CHUNK_W  # 462 (positions [59, 3305))

COMPUTE_DT = mybir.dt.float16

BLOCK_SIZE = 32
QMAX = 31.0
QMIN = -32.0

_CACHE = {}


def _axs6_dequant_np(w: np.ndarray) -> np.ndarray:
    """Bit-identical numpy replica of the reference jax axs6_dequant."""
    shape = w.shape
    wf = w.reshape(-1, BLOCK_SIZE).astype(np.float32)
    scale = (np.max(np.abs(wf), axis=1, keepdims=True) / np.float32(QMAX)).astype(
        np.float32
    )
    scale = np.where(scale == 0, np.float32(1.0), scale)
    q = np.clip(np.round(wf / scale), np.float32(QMIN), np.float32(QMAX))
    return (q * scale).reshape(shape).astype(np.float32)


def _build_module():
    nc = bacc.Bacc()
    x_d = nc.declare_dram_parameter(
        "x", [BP, CIN, HP, WP], mybir.dt.float32, isOutput=False
    )
    w_d = nc.declare_dram_parameter(
        "w", [NCHUNK, 128, KH * KW * NQ * 128], mybir.dt.float32, isOutput=False
    )
    b_d = nc.declare_dram_parameter("bias", [128, NQ], mybir.dt.float32, isOutput=False)
    o_d = nc.declare_dram_parameter(
        "out", [BP, COUT, H, W], mybir.dt.float32, isOutput=True
    )

    with TileContext(nc) as tc:
        with (
            tc.tile_pool(name="persist", bufs=1) as persist,
            tc.tile_pool(name="stage", bufs=3) as stage_pool,
            tc.tile_pool(name="psum", bufs=4, space="PSUM") as psum_pool,
        ):
            bias_sb = persist.tile([128, NQ], mybir.dt.float32, tag="bias")
            nc.sync.dma_start(out=bias_sb, in_=b_d[:, :])

            wt = []
            for c in range(NCHUNK):
                t = persist.tile(
                    [128, KH * KW * NQ * 128], COMPUTE_DT, tag=f"wt{c}"
                )
                nc.gpsimd.dma_start(out=t, in_=w_d[c, :, :])
                wt.append(t)

            xs = [[None] * BP for _ in range(NCHUNK)]
            for n in range(BP):
                for c in range(NCHUNK):
                    t = persist.tile([128, PLANE], COMPUTE_DT, tag=f"xs{c}_{n}")
                    nc.gpsimd.dma_start(
                        out=t,
                        in_=x_d[n, c * 128 : (c + 1) * 128, :, :].rearrange(
                            "c h w -> c (h w)"
                        ),
                    )
                    xs[c][n] = t

            for n in range(BP):
                for q in range(NQ):
                    stage = stage_pool.tile([128, OUT_SPAN], mybir.dt.float32)
                    for k in range(N_K):
                        w_k = CHUNK_W if k < N_K - 1 else LAST_W
                        ps = psum_pool.tile([128, CHUNK_W], mybir.dt.float32)
                        j = 0
                        for c in range(NCHUNK):
                            for tap in range(KH * KW):
                                dh, dw = tap // 3, tap % 3
                                off = (dh - 1) * WP + (dw - 1)
                                base = 59 + k * CHUNK_W + off
                                nc.tensor.matmul(
                                    ps[:, :w_k],
                                    wt[c][:, (tap * NQ + q) * 128 : (tap * NQ + q + 1) * 128],
                                    xs[c][n][:, base : base + w_k],
                                    start=(j == 0),
                                    stop=(j == 2 * KH * KW - 1),
                                )
                                j += 1
                        nc.vector.tensor_scalar_add(
                            out=stage[:, k * CHUNK_W : k * CHUNK_W + w_k],
                            in0=ps[:, :w_k],
                            scalar1=bias_sb[:, q : q + 1],
                        )
                    src = stage[:, :].rearrange("p (h w) -> p h w", w=WP)[:, :, 0:W]
                    nc.sync.dma_start(
                        out=o_d[n, q * 128 : (q + 1) * 128, :, :], in_=src
                    )
    nc.compile()
    return nc


def _get_module():
    if "nc" not in _CACHE:
        _CACHE["nc"] = _build_module()
    return _CACHE["nc"]


def kernel(x: np.ndarray, weight: np.ndarray, bias: np.ndarray) -> np.ndarray:
    x = np.ascontiguousarray(x, dtype=np.float32)
    weight = np.ascontiguousarray(weight, dtype=np.float32)
    bias = np.ascontiguousarray(bias, dtype=np.float32)

    wdq = _axs6_dequant_np(weight)  # [COUT, CIN, 3, 3]
    # lhsT layout: [chunk, ci, ((tap*NQ)+q)*128 + co]
    w6 = wdq.reshape(NQ, 128, NCHUNK, 128, KH * KW)  # [q, co, c, ci, tap]
    w_lhsT = np.ascontiguousarray(
        w6.transpose(2, 3, 4, 0, 1).reshape(NCHUNK, 128, KH * KW * NQ * 128)
    )
    bias_h = np.ascontiguousarray(bias.reshape(NQ, 128).T)  # [co, q]

    x_pad = np.zeros((B, CIN, HP, WP), dtype=np.float32)
    x_pad[:, :, 1 : H + 1, 1 : W + 1] = x

    in_maps = [
        {"x": x_pad[i * BP : (i + 1) * BP], "w": w_lhsT, "bias": bias_h}
        for i in range(N_CORES)
    ]
    nc = _get_module()
    res = run_bass_kernel_spmd(nc, in_maps, core_ids=list(range(N_CORES)))
    return np.concatenate([r["out"] for r in res.results], axis=0)


# revision 5
# speedup vs baseline: 1.2359x; 1.2359x over previous
"""AXSConv2d (6-bit block fake-quant 3x3 conv, stride 1, pad 1) on 8 trn2
NeuronCores.

Sharding: data-parallel over batch (32 images -> 4 per core); dequantized
weight + bias replicated on every core.

Device algorithm (per core):
  - Input x is zero-padded on host to [4, 256, 58, 58] so that the conv
    becomes 9 constant-offset shifted matmuls over the flattened padded
    plane (3364 = 58*58 positions).
  - SBUF holds x as fp16 [ci(128) x (chunk, n, 3364)] (2 ci-chunks).
  - For each (n, co_tile of 128, spatial chunk of <=464 cols): accumulate
    18 matmuls (2 ci-chunks x 9 taps) into one PSUM bank, evict with a
    per-partition bias add (DVE) into an SBUF staging tile, then DMA the
    valid 56x56 interior back to HBM.
  - Compute dtype fp16 (PSUM accumulation fp32): ~3e-4 rel err vs fp32
    reference, full TensorE rate.

The weight fake-quant (axs6) is computed on host in fp32 — bit-identical
to the reference jax math — so device error is matmul rounding only.
"""
import sys

import numpy as np

for _p in ("/opt/trn_rl_repo", "/opt/pypackages"):
    if _p not in sys.path:
        sys.path.append(_p)

import concourse.bass as bass  # noqa: E402
import concourse.mybir as mybir  # noqa: E402
from concourse import bacc  # noqa: E402
from concourse import bass_utils  # noqa: E402
from concourse.tile import TileContext  # noqa: E402
from concourse.bass_utils import run_bass_kernel_spmd  # noqa: E402

# All 2016 matmuls here carry distinct weights, so walrus's ldw-opt (skip /
# background-buffer redundant weight loads) is pure win; bass_utils pins it
# off, so rewrite the flag on the walrus command line.
LDW_OPT = False  # walrus rejects explicit InstLdweights with ldw-opt enabled
if LDW_OPT and not getattr(bass_utils.run_command, "_ldw_opt_patch", False):
    _orig_run_command = bass_utils.run_command

    def _run_command_ldw(argv, **kwargs):
        argv = [
            a.replace("--enable-ldw-opt=false", "--enable-ldw-opt=true")
            if isinstance(a, str)
            else a
            for a in argv
        ]
        return _orig_run_command(argv, **kwargs)

    _run_command_ldw._ldw_opt_patch = True
    bass_utils.run_command = _run_command_ldw

# Problem shape (hardcoded: the harness contract for this problem)
N_CORES = 8
B, CIN, COUT, H, W = 32, 256, 512, 56, 56
KH = KW = 3
BP = B // N_CORES          # batch per core
HP, WP = H + 2, W + 2      # padded plane
PLANE = HP * WP            # 3364
NCHUNK = CIN // 128        # ci chunks of 128
NQ = COUT // 128           # co tiles of 128
OUT_SPAN = H * WP          # 3248: flat positions [59, 59+3248) hold all outputs
CHUNK_W = 464              # matmul moving width (<= 512 fp32 psum bank)
N_K = 7                    # ceil(3246 / 464): 6 x 464 + 462
LAST_W = (OUT_SPAN - 2) - (N_K - 1) * CHUNK_W  # 462 (positions [59, 3305))

COMPUTE_DT = mybir.dt.float16

BLOCK_SIZE = 32
QMAX = 31.0
QMIN = -32.0

_CACHE = {}


def _axs6_dequant_np(w: np.ndarray) -> np.ndarray:
    """Bit-identical numpy replica of the reference jax axs6_dequant."""
    shape = w.shape
    wf = w.reshape(-1, BLOCK_SIZE).astype(np.float32)
    scale = (np.max(np.abs(wf), axis=1, keepdims=True) / np.float32(QMAX)).astype(
        np.float32
    )
    scale = np.where(scale == 0, np.float32(1.0), scale)
    q = np.clip(np.round(wf / scale), np.float32(QMIN), np.float32(QMAX))
    return (q * scale).reshape(shape).astype(np.float32)


def _build_module():
    nc = bacc.Bacc()
    x_d = nc.declare_dram_parameter(
        "x", [BP, CIN, HP, WP], COMPUTE_DT, isOutput=False
    )
    w_d = nc.declare_dram_parameter(
        "w", [NCHUNK, NQ, 128, KH * KW * 128], COMPUTE_DT, isOutput=False
    )
    b_d = nc.declare_dram_parameter("bias", [128, NQ], mybir.dt.float32, isOutput=False)
    o_d = nc.declare_dram_parameter(
        "out", [BP, COUT, H, W], mybir.dt.float32, isOutput=True
    )

    with TileContext(nc) as tc:
        with (
            tc.tile_pool(name="persist", bufs=1) as persist,
            tc.tile_pool(name="stage", bufs=3) as stage_pool,
            tc.tile_pool(name="psum", bufs=4, space="PSUM") as psum_pool,
        ):
            bias_sb = persist.tile([128, NQ], mybir.dt.float32, tag="bias")
            nc.sync.dma_start(out=bias_sb, in_=b_d[:, :])

            # issue order matters: the first compute group (n=0, q=0) needs
            # xs[*][0] and wt[*][0] only, so those DMAs go first and compute
            # overlaps the rest of the loads.
            xs = [[None] * BP for _ in range(NCHUNK)]
            wt = [[None] * NQ for _ in range(NCHUNK)]

            def load_x(c, n):
                t = persist.tile([128, PLANE], COMPUTE_DT, tag=f"xs{c}_{n}")
                nc.sync.dma_start(
                    out=t,
                    in_=x_d[n, c * 128 : (c + 1) * 128, :, :].rearrange(
                        "c h w -> c (h w)"
                    ),
                )
                xs[c][n] = t

            def load_w(c, q):
                t = persist.tile([128, KH * KW * 128], COMPUTE_DT, tag=f"wt{c}_{q}")
                nc.sync.dma_start(out=t, in_=w_d[c, q, :, :])
                wt[c][q] = t

            for c in range(NCHUNK):
                load_x(c, 0)
            for c in range(NCHUNK):
                load_w(c, 0)
            for q in range(1, NQ):
                for c in range(NCHUNK):
                    load_w(c, q)
            for n in range(1, BP):
                for c in range(NCHUNK):
                    load_x(c, n)

            for n in range(BP):
                for q in range(NQ):
                    stage = stage_pool.tile([128, OUT_SPAN], mybir.dt.float32)
                    for k in range(N_K):
                        w_k = CHUNK_W if k < N_K - 1 else LAST_W
                        ps = psum_pool.tile([128, CHUNK_W], mybir.dt.float32)
                        j = 0
                        for c in range(NCHUNK):
                            for tap in range(KH * KW):
                                dh, dw = tap // 3, tap % 3
                                off = (dh - 1) * WP + (dw - 1)
                                base = 59 + k * CHUNK_W + off
                                nc.tensor.matmul(
                                    ps[:, :w_k],
                                    wt[c][q][:, tap * 128 : (tap + 1) * 128],
                                    xs[c][n][:, base : base + w_k],
                                    start=(j == 0),
                                    stop=(j == 2 * KH * KW - 1),
                                )
                                j += 1
                        nc.vector.tensor_scalar_add(
                            out=stage[:, k * CHUNK_W : k * CHUNK_W + w_k],
                            in0=ps[:, :w_k],
                            scalar1=bias_sb[:, q : q + 1],
                        )
                        done = k * CHUNK_W + w_k  # completed stage cols
                        h_done = done // WP       # complete rows available
                        h_sent = (k * CHUNK_W) // WP if k else 0
                        if k == N_K - 1:
                            h_done = H
                        if h_done > h_sent:
                            sv = stage[:, :].rearrange("p (h w) -> p h w", w=WP)
                            nc.sync.dma_start(
                                out=o_d[n, q * 128 : (q + 1) * 128, h_sent:h_done, :],
                                in_=sv[:, h_sent:h_done, 0:W],
                            )
    nc.compile()
    return nc


def _get_module():
    if "nc" not in _CACHE:
        _CACHE["nc"] = _build_module()
    return _CACHE["nc"]


def kernel(x: np.ndarray, weight: np.ndarray, bias: np.ndarray) -> np.ndarray:
    x = np.ascontiguousarray(x, dtype=np.float32)
    weight = np.ascontiguousarray(weight, dtype=np.float32)
    bias = np.ascontiguousarray(bias, dtype=np.float32)

    wdq = _axs6_dequant_np(weight)  # [COUT, CIN, 3, 3]
    # lhsT layout: [chunk, q, ci, tap*128 + co]
    w6 = wdq.reshape(NQ, 128, NCHUNK, 128, KH * KW)  # [q, co, c, ci, tap]
    w_lhsT = np.ascontiguousarray(
        w6.transpose(2, 0, 3, 4, 1).reshape(NCHUNK, NQ, 128, KH * KW * 128)
    ).astype(np.float16)
    bias_h = np.ascontiguousarray(bias.reshape(NQ, 128).T)  # [co, q]

    x_pad = np.zeros((B, CIN, HP, WP), dtype=np.float16)
    x_pad[:, :, 1 : H + 1, 1 : W + 1] = x.astype(np.float16)

    in_maps = [
        {"x": x_pad[i * BP : (i + 1) * BP], "w": w_lhsT, "bias": bias_h}
        for i in range(N_CORES)
    ]
    nc = _get_module()
    res = run_bass_kernel_spmd(nc, in_maps, core_ids=list(range(N_CORES)))
    return np.concatenate([r["out"] for r in res.results], axis=0)


# revision 6
# speedup vs baseline: 1.2633x; 1.0221x over previous
"""AXSConv2d (6-bit block fake-quant 3x3 conv, stride 1, pad 1) on 8 trn2
NeuronCores.

Sharding: data-parallel over batch (32 images -> 4 per core); dequantized
weight + bias replicated on every core.

Device algorithm (per core):
  - Input x is zero-padded on host to [4, 256, 58, 58] so that the conv
    becomes 9 constant-offset shifted matmuls over the flattened padded
    plane (3364 = 58*58 positions).
  - SBUF holds x as fp16 [ci(128) x (chunk, n, 3364)] (2 ci-chunks).
  - For each (n, co_tile of 128, spatial chunk of <=464 cols): accumulate
    18 matmuls (2 ci-chunks x 9 taps) into one PSUM bank, evict with a
    per-partition bias add (DVE) into an SBUF staging tile, then DMA the
    valid 56x56 interior back to HBM.
  - Compute dtype fp16 (PSUM accumulation fp32): ~3e-4 rel err vs fp32
    reference, full TensorE rate.

The weight fake-quant (axs6) is computed on host in fp32 — bit-identical
to the reference jax math — so device error is matmul rounding only.
"""
import sys

import numpy as np

for _p in ("/opt/trn_rl_repo", "/opt/pypackages"):
    if _p not in sys.path:
        sys.path.append(_p)

import concourse.bass as bass  # noqa: E402
import concourse.mybir as mybir  # noqa: E402
from concourse import bacc  # noqa: E402
from concourse import bass_utils  # noqa: E402
from concourse.tile import TileContext  # noqa: E402
from concourse.bass_utils import run_bass_kernel_spmd  # noqa: E402

# All 2016 matmuls here carry distinct weights, so walrus's ldw-opt (skip /
# background-buffer redundant weight loads) is pure win; bass_utils pins it
# off, so rewrite the flag on the walrus command line.
LDW_OPT = False  # walrus rejects explicit InstLdweights with ldw-opt enabled
if LDW_OPT and not getattr(bass_utils.run_command, "_ldw_opt_patch", False):
    _orig_run_command = bass_utils.run_command

    def _run_command_ldw(argv, **kwargs):
        argv = [
            a.replace("--enable-ldw-opt=false", "--enable-ldw-opt=true")
            if isinstance(a, str)
            else a
            for a in argv
        ]
        return _orig_run_command(argv, **kwargs)

    _run_command_ldw._ldw_opt_patch = True
    bass_utils.run_command = _run_command_ldw

# Problem shape (hardcoded: the harness contract for this problem)
N_CORES = 8
B, CIN, COUT, H, W = 32, 256, 512, 56, 56
KH = KW = 3
BP = B // N_CORES          # batch per core
HP, WP = H + 2, W + 2      # padded plane
PLANE = HP * WP            # 3364
NCHUNK = CIN // 128        # ci chunks of 128
NQ = COUT // 128           # co tiles of 128
R_CHUNK = 8                # output rows per matmul chunk
N_K = H // R_CHUNK         # 7 chunks of 8 rows
CHUNK_W = R_CHUNK * W      # 448 moving cols per matmul (<=512 psum bank)

COMPUTE_DT = mybir.dt.float16

BLOCK_SIZE = 32
QMAX = 31.0
QMIN = -32.0

_CACHE = {}


def _axs6_dequant_np(w: np.ndarray) -> np.ndarray:
    """Bit-identical numpy replica of the reference jax axs6_dequant."""
    shape = w.shape
    wf = w.reshape(-1, BLOCK_SIZE).astype(np.float32)
    scale = (np.max(np.abs(wf), axis=1, keepdims=True) / np.float32(QMAX)).astype(
        np.float32
    )
    scale = np.where(scale == 0, np.float32(1.0), scale)
    q = np.clip(np.round(wf / scale), np.float32(QMIN), np.float32(QMAX))
    return (q * scale).reshape(shape).astype(np.float32)


def _build_module():
    nc = bacc.Bacc()
    x_d = nc.declare_dram_parameter(
        "x", [BP, CIN, HP, WP], COMPUTE_DT, isOutput=False
    )
    w_d = nc.declare_dram_parameter(
        "w", [NCHUNK, NQ, 128, KH * KW * 128], COMPUTE_DT, isOutput=False
    )
    b_d = nc.declare_dram_parameter("bias", [128, NQ], mybir.dt.float32, isOutput=False)
    o_d = nc.declare_dram_parameter(
        "out", [BP, COUT, H, W], mybir.dt.float32, isOutput=True
    )

    with TileContext(nc) as tc:
        with (
            tc.tile_pool(name="persist", bufs=1) as persist,
            tc.tile_pool(name="stage", bufs=3) as stage_pool,
            tc.tile_pool(name="psum", bufs=4, space="PSUM") as psum_pool,
        ):
            bias_sb = persist.tile([128, NQ], mybir.dt.float32, tag="bias")
            nc.sync.dma_start(out=bias_sb, in_=b_d[:, :])

            # issue order matters: the first compute group (n=0, q=0) needs
            # xs[*][0] and wt[*][0] only, so those DMAs go first and compute
            # overlaps the rest of the loads.
            xs = [[None] * BP for _ in range(NCHUNK)]
            wt = [[None] * NQ for _ in range(NCHUNK)]

            def load_x(c, n):
                t = persist.tile([128, PLANE], COMPUTE_DT, tag=f"xs{c}_{n}")
                nc.sync.dma_start(
                    out=t,
                    in_=x_d[n, c * 128 : (c + 1) * 128, :, :].rearrange(
                        "c h w -> c (h w)"
                    ),
                )
                xs[c][n] = t

            def load_w(c, q):
                t = persist.tile([128, KH * KW * 128], COMPUTE_DT, tag=f"wt{c}_{q}")
                nc.sync.dma_start(out=t, in_=w_d[c, q, :, :])
                wt[c][q] = t

            for c in range(NCHUNK):
                load_x(c, 0)
            for c in range(NCHUNK):
                load_w(c, 0)
            for q in range(1, NQ):
                for c in range(NCHUNK):
                    load_w(c, q)
            for n in range(1, BP):
                for c in range(NCHUNK):
                    load_x(c, n)

            for n in range(BP):
                for q in range(NQ):
                    stage = stage_pool.tile([128, H * W], mybir.dt.float32)
                    xv = [
                        xs[c][n][:, :].rearrange("p (h w) -> p h w", w=WP)
                        for c in range(NCHUNK)
                    ]
                    for k in range(N_K):
                        h0 = k * R_CHUNK
                        ps = psum_pool.tile([128, CHUNK_W], mybir.dt.float32)
                        j = 0
                        for c in range(NCHUNK):
                            for tap in range(KH * KW):
                                dh, dw = tap // 3, tap % 3
                                nc.tensor.matmul(
                                    ps[:, :],
                                    wt[c][q][:, tap * 128 : (tap + 1) * 128],
                                    xv[c][:, h0 + dh : h0 + dh + R_CHUNK, dw : dw + W],
                                    start=(j == 0),
                                    stop=(j == 2 * KH * KW - 1),
                                )
                                j += 1
                        nc.vector.tensor_scalar_add(
                            out=stage[:, k * CHUNK_W : (k + 1) * CHUNK_W],
                            in0=ps[:, :],
                            scalar1=bias_sb[:, q : q + 1],
                        )
                        nc.sync.dma_start(
                            out=o_d[n, q * 128 : (q + 1) * 128, h0 : h0 + R_CHUNK, :],
                            in_=stage[:, k * CHUNK_W : (k + 1) * CHUNK_W],
                        )
    nc.compile()
    return nc


def _get_module():
    if "nc" not in _CACHE:
        _CACHE["nc"] = _build_module()
    return _CACHE["nc"]


def kernel(x: np.ndarray, weight: np.ndarray, bias: np.ndarray) -> np.ndarray:
    x = np.ascontiguousarray(x, dtype=np.float32)
    weight = np.ascontiguousarray(weight, dtype=np.float32)
    bias = np.ascontiguousarray(bias, dtype=np.float32)

    wdq = _axs6_dequant_np(weight)  # [COUT, CIN, 3, 3]
    # lhsT layout: [chunk, q, ci, tap*128 + co]
    w6 = wdq.reshape(NQ, 128, NCHUNK, 128, KH * KW)  # [q, co, c, ci, tap]
    w_lhsT = np.ascontiguousarray(
        w6.transpose(2, 0, 3, 4, 1).reshape(NCHUNK, NQ, 128, KH * KW * 128)
    ).astype(np.float16)
    bias_h = np.ascontiguousarray(bias.reshape(NQ, 128).T)  # [co, q]

    x_pad = np.zeros((B, CIN, HP, WP), dtype=np.float16)
    x_pad[:, :, 1 : H + 1, 1 : W + 1] = x.astype(np.float16)

    in_maps = [
        {"x": x_pad[i * BP : (i + 1) * BP], "w": w_lhsT, "bias": bias_h}
        for i in range(N_CORES)
    ]
    nc = _get_module()
    res = run_bass_kernel_spmd(nc, in_maps, core_ids=list(range(N_CORES)))
    return np.concatenate([r["out"] for r in res.results], axis=0)


# revision 7
# speedup vs baseline: 1.2823x; 1.0150x over previous
"""AXSConv2d (6-bit block fake-quant 3x3 conv, stride 1, pad 1) on 8 trn2
NeuronCores.

Sharding: data-parallel over batch (32 images -> 4 per core); dequantized
weight + bias replicated on every core.

Device algorithm (per core):
  - Input x is zero-padded on host to [4, 256, 58, 58] so that the conv
    becomes 9 constant-offset shifted matmuls over the flattened padded
    plane (3364 = 58*58 positions).
  - SBUF holds x as fp16 [ci(128) x (chunk, n, 3364)] (2 ci-chunks).
  - For each (n, co_tile of 128, spatial chunk of <=464 cols): accumulate
    18 matmuls (2 ci-chunks x 9 taps) into one PSUM bank, evict with a
    per-partition bias add (DVE) into an SBUF staging tile, then DMA the
    valid 56x56 interior back to HBM.
  - Compute dtype fp16 (PSUM accumulation fp32): ~3e-4 rel err vs fp32
    reference, full TensorE rate.

The weight fake-quant (axs6) is computed on host in fp32 — bit-identical
to the reference jax math — so device error is matmul rounding only.
"""
import sys

import numpy as np

for _p in ("/opt/trn_rl_repo", "/opt/pypackages"):
    if _p not in sys.path:
        sys.path.append(_p)

import concourse.bass as bass  # noqa: E402
import concourse.mybir as mybir  # noqa: E402
from concourse import bacc  # noqa: E402
from concourse import bass_utils  # noqa: E402
from concourse.tile import TileContext  # noqa: E402
from concourse.bass_utils import run_bass_kernel_spmd  # noqa: E402

# All 2016 matmuls here carry distinct weights, so walrus's ldw-opt (skip /
# background-buffer redundant weight loads) is pure win; bass_utils pins it
# off, so rewrite the flag on the walrus command line.
LDW_OPT = False  # walrus rejects explicit InstLdweights with ldw-opt enabled
if LDW_OPT and not getattr(bass_utils.run_command, "_ldw_opt_patch", False):
    _orig_run_command = bass_utils.run_command

    def _run_command_ldw(argv, **kwargs):
        argv = [
            a.replace("--enable-ldw-opt=false", "--enable-ldw-opt=true")
            if isinstance(a, str)
            else a
            for a in argv
        ]
        return _orig_run_command(argv, **kwargs)

    _run_command_ldw._ldw_opt_patch = True
    bass_utils.run_command = _run_command_ldw

# Problem shape (hardcoded: the harness contract for this problem)
N_CORES = 8
B, CIN, COUT, H, W = 32, 256, 512, 56, 56
KH = KW = 3
BP = B // N_CORES          # batch per core
HP, WP = H + 2, W + 2      # padded plane
PLANE = HP * WP            # 3364
NCHUNK = CIN // 128        # ci chunks of 128
NQ = COUT // 128           # co tiles of 128
R_CHUNK = 8                # output rows per matmul chunk
N_K = H // R_CHUNK         # 7 chunks of 8 rows
CHUNK_W = R_CHUNK * W      # 448 moving cols per matmul (<=512 psum bank)

COMPUTE_DT = mybir.dt.float16

BLOCK_SIZE = 32
QMAX = 31.0
QMIN = -32.0

_CACHE = {}


def _axs6_dequant_np(w: np.ndarray) -> np.ndarray:
    """Bit-identical numpy replica of the reference jax axs6_dequant."""
    shape = w.shape
    wf = w.reshape(-1, BLOCK_SIZE).astype(np.float32)
    scale = (np.max(np.abs(wf), axis=1, keepdims=True) / np.float32(QMAX)).astype(
        np.float32
    )
    scale = np.where(scale == 0, np.float32(1.0), scale)
    q = np.clip(np.round(wf / scale), np.float32(QMIN), np.float32(QMAX))
    return (q * scale).reshape(shape).astype(np.float32)


def _build_module():
    nc = bacc.Bacc()
    x_d = nc.declare_dram_parameter(
        "x", [BP, CIN, HP, WP], COMPUTE_DT, isOutput=False
    )
    w_d = nc.declare_dram_parameter(
        "w", [NCHUNK, NQ, 128, KH * KW * 128], COMPUTE_DT, isOutput=False
    )
    b_d = nc.declare_dram_parameter("bias", [128, NQ], mybir.dt.float32, isOutput=False)
    o_d = nc.declare_dram_parameter(
        "out", [BP, COUT, H, W], mybir.dt.float32, isOutput=True
    )

    with TileContext(nc) as tc:
        with (
            tc.tile_pool(name="persist", bufs=1) as persist,
            tc.tile_pool(name="stage", bufs=3) as stage_pool,
            tc.tile_pool(name="psum", bufs=8, space="PSUM") as psum_pool,
        ):
            bias_sb = persist.tile([128, NQ], mybir.dt.float32, tag="bias")
            nc.sync.dma_start(out=bias_sb, in_=b_d[:, :])

            # issue order matters: the first compute group (n=0, q=0) needs
            # xs[*][0] and wt[*][0] only, so those DMAs go first and compute
            # overlaps the rest of the loads.
            # x is split per image into a 10-row head (rows 0..9, feeds the
            # k=0 chunk) and a 50-row tail (rows 8..57, feeds k>=1), so the
            # first matmul group only waits on ~0.9MB of DMA.
            HEAD_R, TAIL_R0 = 10, 8

            def load_w(c, q):
                t = persist.tile([128, KH * KW * 128], COMPUTE_DT, tag=f"wt{c}_{q}")
                nc.sync.dma_start(out=t, in_=w_d[c, q, :, :])
                wt[c][q] = t

            xh = [[None] * BP for _ in range(NCHUNK)]
            xt = [[None] * BP for _ in range(NCHUNK)]
            wt = [[None] * NQ for _ in range(NCHUNK)]

            def load_x_head(c, n):
                t = persist.tile([128, HEAD_R * WP], COMPUTE_DT, tag=f"xh{c}_{n}")
                nc.sync.dma_start(
                    out=t,
                    in_=x_d[n, c * 128 : (c + 1) * 128, 0:HEAD_R, :].rearrange(
                        "c h w -> c (h w)"
                    ),
                )
                xh[c][n] = t

            def load_x_tail(c, n):
                t = persist.tile(
                    [128, (HP - TAIL_R0) * WP], COMPUTE_DT, tag=f"xt{c}_{n}"
                )
                nc.sync.dma_start(
                    out=t,
                    in_=x_d[n, c * 128 : (c + 1) * 128, TAIL_R0:HP, :].rearrange(
                        "c h w -> c (h w)"
                    ),
                )
                xt[c][n] = t

            for c in range(NCHUNK):
                load_x_head(c, 0)
            for c in range(NCHUNK):
                load_w(c, 0)
            for c in range(NCHUNK):
                load_x_tail(c, 0)
            for q in range(1, NQ):
                for c in range(NCHUNK):
                    load_w(c, q)
            for n in range(1, BP):
                for c in range(NCHUNK):
                    load_x_head(c, n)
                    load_x_tail(c, n)

            for n in range(BP):
                for q in range(NQ):
                    stage = stage_pool.tile([128, H * W], mybir.dt.float32)
                    xhv = [
                        xh[c][n][:, :].rearrange("p (h w) -> p h w", w=WP)
                        for c in range(NCHUNK)
                    ]
                    xtv = [
                        xt[c][n][:, :].rearrange("p (h w) -> p h w", w=WP)
                        for c in range(NCHUNK)
                    ]
                    for k in range(N_K):
                        h0 = k * R_CHUNK
                        ps = psum_pool.tile([128, CHUNK_W], mybir.dt.float32)
                        j = 0
                        for c in range(NCHUNK):
                            for tap in range(KH * KW):
                                dh, dw = tap // 3, tap % 3
                                if k == 0:
                                    rhs = xhv[c][:, dh : dh + R_CHUNK, dw : dw + W]
                                else:
                                    r0 = h0 + dh - TAIL_R0
                                    rhs = xtv[c][:, r0 : r0 + R_CHUNK, dw : dw + W]
                                nc.tensor.matmul(
                                    ps[:, :],
                                    wt[c][q][:, tap * 128 : (tap + 1) * 128],
                                    rhs,
                                    start=(j == 0),
                                    stop=(j == 2 * KH * KW - 1),
                                )
                                j += 1
                        nc.vector.tensor_scalar_add(
                            out=stage[:, k * CHUNK_W : (k + 1) * CHUNK_W],
                            in0=ps[:, :],
                            scalar1=bias_sb[:, q : q + 1],
                        )
                        nc.sync.dma_start(
                            out=o_d[n, q * 128 : (q + 1) * 128, h0 : h0 + R_CHUNK, :],
                            in_=stage[:, k * CHUNK_W : (k + 1) * CHUNK_W],
                        )
    nc.compile()
    return nc


def _get_module():
    if "nc" not in _CACHE:
        _CACHE["nc"] = _build_module()
    return _CACHE["nc"]


def kernel(x: np.ndarray, weight: np.ndarray, bias: np.ndarray) -> np.ndarray:
    x = np.ascontiguousarray(x, dtype=np.float32)
    weight = np.ascontiguousarray(weight, dtype=np.float32)
    bias = np.ascontiguousarray(bias, dtype=np.float32)

    wdq = _axs6_dequant_np(weight)  # [COUT, CIN, 3, 3]
    # lhsT layout: [chunk, q, ci, tap*128 + co]
    w6 = wdq.reshape(NQ, 128, NCHUNK, 128, KH * KW)  # [q, co, c, ci, tap]
    w_lhsT = np.ascontiguousarray(
        w6.transpose(2, 0, 3, 4, 1).reshape(NCHUNK, NQ, 128, KH * KW * 128)
    ).astype(np.float16)
    bias_h = np.ascontiguousarray(bias.reshape(NQ, 128).T)  # [co, q]

    x_pad = np.zeros((B, CIN, HP, WP), dtype=np.float16)
    x_pad[:, :, 1 : H + 1, 1 : W + 1] = x.astype(np.float16)

    in_maps = [
        {"x": x_pad[i * BP : (i + 1) * BP], "w": w_lhsT, "bias": bias_h}
        for i in range(N_CORES)
    ]
    nc = _get_module()
    res = run_bass_kernel_spmd(nc, in_maps, core_ids=list(range(N_CORES)))
    return np.concatenate([r["out"] for r in res.results], axis=0)
